# revision 31
# baseline (speedup 1.0000x reference)
"""Trainium2 Bass kernel for the low-rank linear operator.

Math: the reference collapses algebraically. With y = linspace(-1,1,H),
x = linspace(-1,1,W), dx = 2/(W-1):

  Sy[b,i] = sum_{h,w} v[b,i,h,w] * y_h ;  Sx[b,i] = sum_{h,w} v * x_w
  inner[b,r] = dx * sum_i (Sy*psi[r,i,0] + Sx*psi[r,i,1])
  A[b,o] = sum_r inner*phi[o,r,0] ;  B[b,o] = sum_r inner*phi[o,r,1]
  u[b,o,h,w] = A[b,o]*y_h + B[b,o]*x_w

Pure data movement + rank-2 output synthesis; the roofline is HBM
traffic, so transfers run in reduced precision (rel-err gate 2e-2):
  - v uploaded as int8 with per-(b,i) scale (SWDGE dma casts to bf16 on
    load); scales are folded into the psi line-tables host-side.
  - u produced as uint8 with per-(b,o) scale s=(|A|+|B|)/126 and +128
    offset (gen writes bf16 tiles; the output DMA casts bf16->u8);
    dequantized on the host.

Input layout: per 16-channel block, tile [128, 32, 256] with partition
p = 8*i_local + hb (hb = h//32): each partition holds 32 contiguous
h-rows of one channel = one 8KB (i8) DMA descriptor.

Reductions: the idle PE pre-folds hb 8->1: matmul with lhsT = indicator/
y0-weighted tables contracts partitions into 128 per-channel "lines"
(32k + 2*il + t; t=0 plain sum, t=1 y0-weighted), 4x less tree work.
Since matmul psum bases are limited to {0,32,64}, blocks 2+3 share one
M=64 accumulating matmul pair with zero-padded lhsT halves. Drained
lines (bf16) feed two DVE halving trees (w-tree -> y-side via affine
y = y0 + dy*hl; h-tree -> x-side), then two K=128 matmuls against
per-line psi tables give inner[b,:] directly; transpose + phicat matmul
give (A,B); PE outer products against [y_even|y_odd|ones] build the
per-partition bias/scale tiles (+128 via an accumulated constant row).

Gen: baseline-style h-pair layout (p = h//2), one tensor_scalar /
activation per (o, hh) rotated across DVE/ACT/Pool.
"""

import sys

try:
    import concourse.bass as bass  # noqa: F401
except ImportError:
    for _p in ("/opt/trn_rl_repo", "/root/.axon_site/_ro/trn_rl_repo"):
        if _p not in sys.path:
            sys.path.insert(0, _p)

import numpy as np
import ml_dtypes

import concourse.bacc as bacc
import concourse.bass as bass
import concourse.mybir as mybir
import concourse.tile as tile
from concourse.bass_utils import run_bass_kernel_spmd

F32 = mybir.dt.float32
F16 = mybir.dt.float16
BF16 = mybir.dt.bfloat16
U8 = mybir.dt.uint8
MULT = mybir.AluOpType.mult
ADD = mybir.AluOpType.add
AXX = mybir.AxisListType.X
IDENT = mybir.ActivationFunctionType.Identity

B, CI, CO, R, H, W = 16, 64, 64, 64, 256, 256
N_CORES = 8
BPC = B // N_CORES

INPUT_I8 = True  # False: bf16 input upload; True: int8 + per-channel scale
GEN_FP16 = True   # gen ops write fp16, output DMA casts to u8 (SWDGE)

IBLK = 16
NBLK = CI // IBLK   # 4 input blocks per batch
OBLK = 8
NOBLK = CO // OBLK  # 8 output DMAs per batch

_GEN_ENGINES = ("dve", "act", "pool")


def build_nc():
    nc = bacc.Bacc("TRN2", target_bir_lowering=False, debug=False)

    vdt = mybir.dt.int8 if INPUT_I8 else BF16
    v = nc.dram_tensor("v", [BPC, CI, H, W], vdt, kind="ExternalInput")
    xw = nc.dram_tensor("xw", [128, W], BF16, kind="ExternalInput")
    xrep = nc.dram_tensor("xrep", [128, W], BF16 if GEN_FP16 else F32, kind="ExternalInput")
    # PE hb-fold tables. ind2[p, 2*il+t] = [p//8==il] * (1 if t==0 else y[32*(p%8)])
    ind2 = nc.dram_tensor("ind2", [128, 32], BF16, kind="ExternalInput")
    ind2a = nc.dram_tensor("ind2a", [128, 64], BF16, kind="ExternalInput")
    ind2b = nc.dram_tensor("ind2b", [128, 64], BF16, kind="ExternalInput")
    # line tables: line p' = 32k + 2il + t <-> channel i = 16k+il; per-batch scaled
    psiYt = nc.dram_tensor("psiYt", [128, BPC, R], F32, kind="ExternalInput")
    psiXt = nc.dram_tensor("psiXt", [128, BPC, R], F32, kind="ExternalInput")
    wy2 = nc.dram_tensor("wy2", [128, 32], F32, kind="ExternalInput")
    phicat = nc.dram_tensor("phicat", [R, 2 * CO], F32, kind="ExternalInput")
    ybc = nc.dram_tensor("ybc", [1, 384], F32, kind="ExternalInput")
    c128 = nc.dram_tensor("c128", [1, 2 * CO], F32, kind="ExternalInput")
    ident1 = nc.dram_tensor("ident1", [1, 1], F32, kind="ExternalInput")
    u8o = nc.dram_tensor("u8o", [BPC, CO, H, W], U8, kind="ExternalOutput")
    s_out = nc.dram_tensor("s_out", [BPC, CO], F32, kind="ExternalOutput")

    with tile.TileContext(nc) as tc:
        with (
            tc.tile_pool(name="consts", bufs=1) as consts,
            tc.tile_pool(name="inp", bufs=8) as in_pool,
            tc.tile_pool(name="tree", bufs=1) as tree_pool,
            tc.tile_pool(name="sm", bufs=2) as small,
            tc.tile_pool(name="syx", bufs=2) as syx_pool,
            tc.tile_pool(name="bc", bufs=6) as bc_pool,
            tc.tile_pool(name="outp", bufs=2) as out_pool,
            tc.tile_pool(name="psumQ", bufs=4, space="PSUM") as psum_q,
            tc.tile_pool(name="psumT", bufs=1, space="PSUM") as psum_t,
            tc.tile_pool(name="psumBC", bufs=2, space="PSUM") as psum_bc,
        ):
            sb_xw = consts.tile([128, W], BF16)
            nc.scalar.dma_start(sb_xw[:], xw[:])
            sb_xrep = consts.tile([128, W], BF16 if GEN_FP16 else F32)
            nc.scalar.dma_start(sb_xrep[:], xrep[:])
            sb_ind2 = consts.tile([128, 32], BF16)
            nc.scalar.dma_start(sb_ind2[:], ind2[:])
            sb_ind2a = consts.tile([128, 64], BF16)
            nc.scalar.dma_start(sb_ind2a[:], ind2a[:])
            sb_ind2b = consts.tile([128, 64], BF16)
            nc.scalar.dma_start(sb_ind2b[:], ind2b[:])
            sb_psiYt = consts.tile([128, BPC, R], F32)
            nc.scalar.dma_start(sb_psiYt[:], psiYt[:])
            sb_psiXt = consts.tile([128, BPC, R], F32)
            nc.scalar.dma_start(sb_psiXt[:], psiXt[:])
            sb_wy2 = consts.tile([128, 32], F32)
            nc.scalar.dma_start(sb_wy2[:], wy2[:])
            sb_phicat = consts.tile([R, 2 * CO], F32)
            nc.scalar.dma_start(sb_phicat[:], phicat[:])
            sb_ybc = consts.tile([1, 384], F32)
            nc.scalar.dma_start(sb_ybc[:], ybc[:])
            sb_c128 = consts.tile([1, 2 * CO], F32)
            nc.scalar.dma_start(sb_c128[:], c128[:])
            sb_id1 = consts.tile([1, 1], F32)
            nc.scalar.dma_start(sb_id1[:], ident1[:])

            in_tiles = {}
            for b in range(BPC):
                for blk in range(NBLK):
                    t = in_pool.tile([128, 32, W], BF16, tag="in")
                    src = v[b, blk * IBLK:(blk + 1) * IBLK, :, :].rearrange(
                        "i (hb hl) w -> (i hb) hl w", hb=8
                    )
                    if INPUT_I8:
                        nc.gpsimd.dma_start(t[:], src)
                    else:
                        nc.sync.dma_start(t[:], src)
                    in_tiles[(b, blk)] = t

            CHUNK = 2
            NCH = 32 // CHUNK

            def reduce_batch(b):
                """PE hb-fold -> Qsb lines [128=(k,il,t), 32, W] -> trees -> SYX."""
                qsb = tree_pool.tile([128, 32, W], BF16, tag="qsb")
                for c in range(NCH):
                    qp = psum_q.tile([128, CHUNK, W], F32, tag="qp")
                    for blk in range(2):
                        nc.tensor.matmul(
                            qp[32 * blk:32 * (blk + 1), :, :], lhsT=sb_ind2[:],
                            rhs=in_tiles[(b, blk)][:, c * CHUNK:(c + 1) * CHUNK, :],
                            start=True, stop=True,
                        )
                    # blocks 2+3 share the [64:128] region (base-96 writes are
                    # not allowed): extended lhsT halves, accumulate pattern
                    nc.tensor.matmul(
                        qp[64:128, :, :], lhsT=sb_ind2a[:],
                        rhs=in_tiles[(b, 2)][:, c * CHUNK:(c + 1) * CHUNK, :],
                        start=True, stop=False,
                    )
                    nc.tensor.matmul(
                        qp[64:128, :, :], lhsT=sb_ind2b[:],
                        rhs=in_tiles[(b, 3)][:, c * CHUNK:(c + 1) * CHUNK, :],
                        start=False, stop=True,
                    )
                    dst = qsb[:, c * CHUNK:(c + 1) * CHUNK, :]
                    if c % 2 == 0:
                        nc.vector.tensor_copy(dst, qp[:])
                    else:
                        nc.scalar.copy(dst, qp[:])
                # w-halving tree (DVE bf16 2x) keeps hl resolution
                q = qsb
                wlen = W
                for lvl in range(5):  # 256 -> 8
                    wlen //= 2
                    qn = tree_pool.tile([128, 32, wlen], BF16, tag=f"q{lvl}")
                    nc.vector.tensor_tensor(
                        out=qn[:], in0=q[:, :, 0:wlen], in1=q[:, :, wlen:2 * wlen],
                        op=ADD,
                    )
                    q = qn
                cq = tree_pool.tile([128, 32], F32, tag="cq")
                nc.vector.tensor_reduce(out=cq[:], in_=q[:], axis=AXX, op=ADD)
                wq = tree_pool.tile([128, 32], F32, tag="wq")
                nc.vector.tensor_tensor(out=wq[:], in0=cq[:], in1=sb_wy2[:], op=MULT)
                syx = syx_pool.tile([128, 2], F32, tag="syx")
                nc.vector.tensor_reduce(out=syx[:, 0:1], in_=wq[:], axis=AXX, op=ADD)
                # h-halving tree (DVE bf16)
                a = qsb
                hlen = 32
                for lvl in range(5):  # 32 -> 1
                    hlen //= 2
                    an = tree_pool.tile([128, hlen, W], BF16, tag=f"a{lvl}")
                    nc.vector.tensor_tensor(
                        out=an[:], in0=a[:, 0:hlen, :], in1=a[:, hlen:2 * hlen, :],
                        op=ADD,
                    )
                    a = an
                wrs = tree_pool.tile([128, W], F32, tag="wrs")
                nc.vector.tensor_tensor(out=wrs[:], in0=a[:, 0, :], in1=sb_xw[:], op=MULT)
                nc.vector.tensor_reduce(out=syx[:, 1:2], in_=wrs[:], axis=AXX, op=ADD)
                return syx

            def tiny_rest(b, syx):
                """syx [128(il,hb), (k,yx)] partials -> bias/scale tiles for gen."""
                inner_ps = psum_t.tile([1, R], F32, tag="tiny")
                nc.tensor.matmul(
                    inner_ps[:], lhsT=syx[:, 0:1], rhs=sb_psiYt[:, b, :],
                    start=True, stop=False,
                )
                nc.tensor.matmul(
                    inner_ps[:], lhsT=syx[:, 1:2], rhs=sb_psiXt[:, b, :],
                    start=False, stop=True,
                )
                inner_sb = small.tile([1, R], F32, tag="ti1")
                nc.vector.tensor_copy(inner_sb[:], inner_ps[:])

                innT_ps = psum_t.tile([R, 1], F32, tag="tiny2")
                nc.tensor.transpose(innT_ps[:], inner_sb[:], sb_id1[:])
                innT = small.tile([R, 1], F32, tag="ti2")
                nc.vector.tensor_copy(innT[:], innT_ps[:])

                ab_ps = psum_t.tile([1, 2 * CO], F32, tag="tiny")
                nc.tensor.matmul(
                    ab_ps[:], lhsT=innT[:], rhs=sb_phicat[:], start=True, stop=True
                )
                ab = small.tile([1, 2 * CO], F32, tag="ti3")
                nc.vector.tensor_copy(ab[:], ab_ps[:])

                # per-channel scale s = (|A|+|B|)/127, inv, and scaled A,B
                absab = small.tile([1, 2 * CO], F32, tag="ti4")
                nc.scalar.activation(absab[:], ab[:], mybir.ActivationFunctionType.Abs)
                av = absab[:].rearrange("a (o t) -> a o t", t=2)
                s127 = small.tile([1, CO], F32, tag="ti5")
                nc.vector.tensor_tensor(
                    out=s127[:].unsqueeze(2), in0=av[:, :, 0:1], in1=av[:, :, 1:2],
                    op=ADD,
                )
                nc.vector.tensor_scalar(
                    out=s127[:], in0=s127[:], scalar1=1.0 / 126.0, scalar2=None,
                    op0=MULT,
                )
                nc.scalar.dma_start(s_out[b:b + 1, :], s127[:])
                invs = small.tile([1, CO], F32, tag="ti6")
                nc.vector.reciprocal(invs[:], s127[:])
                abq = small.tile([1, 2 * CO], F32, tag="ti7")
                nc.vector.tensor_tensor(
                    out=abq[:].rearrange("a (o t) -> a o t", t=2),
                    in0=ab[:].rearrange("a (o t) -> a o t", t=2),
                    in1=invs[:].unsqueeze(2).broadcast_to([1, CO, 2]),
                    op=MULT,
                )

                outs = []
                for k in range(3):  # bias_even, bias_odd, scale
                    ps = psum_bc.tile([128, 2 * CO], F32, tag="bc")
                    nc.tensor.matmul(
                        ps[:], lhsT=sb_ybc[0:1, 128 * k:128 * (k + 1)], rhs=abq[:],
                        start=True, stop=(k == 2),
                    )
                    if k < 2:  # + 128 offset on bias tiles
                        nc.tensor.matmul(
                            ps[:], lhsT=sb_ybc[0:1, 256:384], rhs=sb_c128[:],
                            start=False, stop=True,
                        )
                    sb = bc_pool.tile([128, 2 * CO], F32, tag="bcs")
                    if k % 2 == 0:
                        nc.scalar.copy(sb[:], ps[:])
                    else:
                        nc.vector.tensor_copy(sb[:], ps[:])
                    outs.append(sb)
                return outs  # [bias_even, bias_odd, scale]

            def gen_batch(b, bias_e, bias_o, scale):
                eng = 0
                for oc in range(NOBLK):
                    ot = out_pool.tile([128, OBLK, 2, W], BF16 if GEN_FP16 else U8, tag="out")
                    for ol in range(OBLK):
                        o = oc * OBLK + ol
                        sc_ap = scale[:, 2 * o + 1:2 * o + 2]
                        for hh in range(2):
                            bias_ap = (bias_e if hh == 0 else bias_o)[:, 2 * o:2 * o + 1]
                            dst = ot[:, ol, hh, :]
                            which = _GEN_ENGINES[eng % len(_GEN_ENGINES)]
                            eng += 1
                            if which == "dve":
                                nc.vector.tensor_scalar(
                                    out=dst, in0=sb_xrep[:], scalar1=sc_ap,
                                    scalar2=bias_ap, op0=MULT, op1=ADD,
                                )
                            elif which == "pool":
                                nc.gpsimd.tensor_scalar(
                                    out=dst, in0=sb_xrep[:], scalar1=sc_ap,
                                    scalar2=bias_ap, op0=MULT, op1=ADD,
                                )
                            else:
                                nc.scalar.activation(
                                    dst, sb_xrep[:], IDENT,
                                    bias=bias_ap, scale=sc_ap,
                                )
                    dma_eng = nc.gpsimd if GEN_FP16 else nc.scalar
                    dma_eng.dma_start(
                        u8o[b, oc * OBLK:(oc + 1) * OBLK, :, :].rearrange(
                            "o (p hh) w -> p o hh w", p=128
                        ),
                        ot[:],
                    )

            syx0 = reduce_batch(0)
            syx1 = reduce_batch(1)
            bc0 = tiny_rest(0, syx0)
            gen_batch(0, *bc0)
            bc1 = tiny_rest(1, syx1)
            gen_batch(1, *bc1)

    nc.compile()
    return nc


def make_in_maps(v, psi, phi):
    y = np.linspace(-1.0, 1.0, H, dtype=np.float64)
    x = np.linspace(-1.0, 1.0, W, dtype=np.float64)
    dx = 2.0 / (W - 1)
    dy = 2.0 / (H - 1)
    bf = ml_dtypes.bfloat16

    p = np.arange(128)
    pp = np.arange(128)  # line index p' = 32k + 2il + t
    il_of = (pp % 32) // 2
    k_of = pp // 32
    t_of = pp % 2
    chan = 16 * k_of + il_of  # [128]

    ind2 = np.zeros((128, 32), np.float64)
    ind2[p, 2 * (p // 8)] = 1.0
    ind2[p, 2 * (p // 8) + 1] = y[32 * (p % 8)]
    ind2a = np.zeros((128, 64), np.float64)
    ind2a[:, 0:32] = ind2
    ind2b = np.zeros((128, 64), np.float64)
    ind2b[:, 32:64] = ind2

    wy2 = np.where(t_of[:, None] == 1, 1.0, dy * np.arange(32)[None, :])

    psiY = np.ascontiguousarray(psi[:, :, 0].T * dx).astype(np.float64)  # [i, r]
    psiX = np.ascontiguousarray(psi[:, :, 1].T * dx).astype(np.float64)
    phicat = np.stack([phi[:, :, 0].T, phi[:, :, 1].T], axis=2).reshape(R, 2 * CO)
    ybc = np.concatenate([y[0::2], y[1::2], np.ones(128)])[None, :].astype(np.float32)
    c128 = np.full((1, 2 * CO), 128.0, dtype=np.float32)

    common = {
        "xw": np.ascontiguousarray(np.broadcast_to(x, (128, W))).astype(bf),
        "xrep": np.ascontiguousarray(np.broadcast_to(x, (128, W))).astype(
            bf if GEN_FP16 else np.float32),
        "ind2": ind2.astype(bf),
        "ind2a": ind2a.astype(bf),
        "ind2b": ind2b.astype(bf),
        "wy2": wy2.astype(np.float32),
        "phicat": np.ascontiguousarray(phicat).astype(np.float32),
        "ybc": ybc,
        "c128": c128,
        "ident1": np.ones((1, 1), dtype=np.float32),
    }

    def psit(sc_b):  # sc_b [BPC, CI] or None -> psiYt/psiXt [128, BPC, R]
        pyt = np.empty((128, BPC, R), np.float64)
        pxt = np.zeros((128, BPC, R), np.float64)
        for b in range(BPC):
            s = sc_b[b] if sc_b is not None else np.ones(CI)
            pyt[:, b, :] = psiY[chan] * s[chan][:, None]
            ev = t_of == 0
            pxt[ev, b, :] = psiX[chan[ev]] * s[chan[ev]][:, None]
        return pyt.astype(np.float32), pxt.astype(np.float32)

    if INPUT_I8:
        vf = v.reshape(N_CORES, BPC, CI, H, W)
        sc = np.abs(vf).max(axis=(3, 4)) / 127.0  # [cores, BPC, CI]
        q = np.rint(vf / sc[..., None, None]).astype(np.int8)
        in_maps = []
        for c in range(N_CORES):
            pyt, pxt = psit(sc[c])
            in_maps.append({"v": q[c], "psiYt": pyt, "psiXt": pxt, **common})
        return in_maps

    pyt, pxt = psit(None)
    common["psiYt"] = pyt
    common["psiXt"] = pxt
    shards = np.ascontiguousarray(v.astype(bf).reshape(N_CORES, BPC, CI, H, W))
    return [{"v": shards[i], **common} for i in range(N_CORES)]


_NC_CACHE = None


def kernel(v, psi, phi):
    global _NC_CACHE
    if _NC_CACHE is None:
        _NC_CACHE = build_nc()
    nc = _NC_CACHE
    in_maps = make_in_maps(
        np.ascontiguousarray(v, dtype=np.float32),
        np.asarray(psi, dtype=np.float32),
        np.asarray(phi, dtype=np.float32),
    )
    res = run_bass_kernel_spmd(nc, in_maps, core_ids=list(range(N_CORES)))
    return postprocess(res.results)


def postprocess(results):
    outs = []
    for r in results:
        u8 = r["u8o"].astype(np.float32)
        s = r["s_out"]  # [BPC, CO]
        u = (u8 - 128.0) * s[:, :, None, None]
        outs.append(u)
    return np.concatenate(outs, axis=0)


if __name__ == "__main__":
    build_nc()
    print("build ok")


# revision 32
# speedup vs baseline: 1.1362x; 1.1362x over previous
"""Trainium2 Bass kernel for the low-rank linear operator.

Math: the reference collapses algebraically. With y = linspace(-1,1,H),
x = linspace(-1,1,W), dx = 2/(W-1):

  Sy[b,i] = sum_{h,w} v[b,i,h,w] * y_h ;  Sx[b,i] = sum_{h,w} v * x_w
  inner[b,r] = dx * sum_i (Sy*psi[r,i,0] + Sx*psi[r,i,1])
  A[b,o] = sum_r inner*phi[o,r,0] ;  B[b,o] = sum_r inner*phi[o,r,1]
  u[b,o,h,w] = A[b,o]*y_h + B[b,o]*x_w

Pure data movement + rank-2 output synthesis; the roofline is HBM
traffic, so transfers run in reduced precision (rel-err gate 2e-2):
  - v uploaded as int8 with per-(b,i) scale (SWDGE dma casts to bf16 on
    load); scales are folded into the psi line-tables host-side.
  - u produced as uint8 with per-(b,o) scale s=(|A|+|B|)/126 and +128
    offset (gen writes bf16 tiles; the output DMA casts bf16->u8);
    dequantized on the host.

Input layout: per 16-channel block, tile [128, 32, 256] with partition
p = 8*i_local + hb (hb = h//32): each partition holds 32 contiguous
h-rows of one channel = one 8KB (i8) DMA descriptor.

Reductions: the idle PE pre-folds hb 8->1: matmul with lhsT = indicator/
y0-weighted tables contracts partitions into 128 per-channel "lines"
(32k + 2*il + t; t=0 plain sum, t=1 y0-weighted), 4x less tree work.
Since matmul psum bases are limited to {0,32,64}, blocks 2+3 share one
M=64 accumulating matmul pair with zero-padded lhsT halves. Drained
lines (bf16) feed two DVE halving trees (w-tree -> y-side via affine
y = y0 + dy*hl; h-tree -> x-side), then two K=128 matmuls against
per-line psi tables give inner[b,:] directly; transpose + phicat matmul
give (A,B); PE outer products against [y_even|y_odd|ones] build the
per-partition bias/scale tiles (+128 via an accumulated constant row).

Gen: baseline-style h-pair layout (p = h//2), one tensor_scalar /
activation per (o, hh) rotated across DVE/ACT/Pool.
"""

import sys

try:
    import concourse.bass as bass  # noqa: F401
except ImportError:
    for _p in ("/opt/trn_rl_repo", "/root/.axon_site/_ro/trn_rl_repo"):
        if _p not in sys.path:
            sys.path.insert(0, _p)

import numpy as np
import ml_dtypes

import concourse.bacc as bacc
import concourse.bass as bass
import concourse.mybir as mybir
import concourse.tile as tile
from concourse.bass_utils import run_bass_kernel_spmd

F32 = mybir.dt.float32
F16 = mybir.dt.float16
BF16 = mybir.dt.bfloat16
U8 = mybir.dt.uint8
MULT = mybir.AluOpType.mult
ADD = mybir.AluOpType.add
AXX = mybir.AxisListType.X
IDENT = mybir.ActivationFunctionType.Identity

B, CI, CO, R, H, W = 16, 64, 64, 64, 256, 256
N_CORES = 8
BPC = B // N_CORES

INPUT_I8 = True  # False: bf16 input upload; True: int8 + per-channel scale
GEN_FP16 = True   # gen ops write fp16, output DMA casts to u8 (SWDGE)

IBLK = 16
NBLK = CI // IBLK   # 4 input blocks per batch
OBLK = 8
NOBLK = CO // OBLK  # 8 output DMAs per batch

_GEN_ENGINES = ("dve", "act", "pool")


def build_nc():
    nc = bacc.Bacc("TRN2", target_bir_lowering=False, debug=False)

    vdt = mybir.dt.int8 if INPUT_I8 else BF16
    v = nc.dram_tensor("v", [BPC, CI, H, W], vdt, kind="ExternalInput")
    xw = nc.dram_tensor("xw", [128, W], BF16, kind="ExternalInput")
    xrep = nc.dram_tensor("xrep", [128, W], BF16 if GEN_FP16 else F32, kind="ExternalInput")
    # PE hb-fold tables. ind2[p, 2*il+t] = [p//8==il] * (1 if t==0 else y[32*(p%8)])
    ind2 = nc.dram_tensor("ind2", [128, 32], BF16, kind="ExternalInput")
    ind2a = nc.dram_tensor("ind2a", [128, 64], BF16, kind="ExternalInput")
    ind2b = nc.dram_tensor("ind2b", [128, 64], BF16, kind="ExternalInput")
    # line tables: line p' = 32k + 2il + t <-> channel i = 16k+il; per-batch scaled
    psiYt = nc.dram_tensor("psiYt", [128, BPC, R], F32, kind="ExternalInput")
    psiXt = nc.dram_tensor("psiXt", [128, BPC, R], F32, kind="ExternalInput")
    wy2 = nc.dram_tensor("wy2", [128, 32], F32, kind="ExternalInput")
    phicat = nc.dram_tensor("phicat", [R, 2 * CO], F32, kind="ExternalInput")
    ybc = nc.dram_tensor("ybc", [1, 384], F32, kind="ExternalInput")
    c128 = nc.dram_tensor("c128", [1, 2 * CO], F32, kind="ExternalInput")
    ident1 = nc.dram_tensor("ident1", [1, 1], F32, kind="ExternalInput")
    u8o = nc.dram_tensor("u8o", [BPC, CO, H, W], U8, kind="ExternalOutput")
    s_out = nc.dram_tensor("s_out", [BPC, CO], F32, kind="ExternalOutput")

    with tile.TileContext(nc) as tc:
        with (
            tc.tile_pool(name="consts", bufs=1) as consts,
            tc.tile_pool(name="inp", bufs=4) as in_pool,
            tc.tile_pool(name="tree", bufs=2) as tree_pool,
            tc.tile_pool(name="sm", bufs=2) as small,
            tc.tile_pool(name="syx", bufs=2) as syx_pool,
            tc.tile_pool(name="bc", bufs=6) as bc_pool,
            tc.tile_pool(name="outp", bufs=4) as out_pool,
            tc.tile_pool(name="psumQ", bufs=4, space="PSUM") as psum_q,
            tc.tile_pool(name="psumT", bufs=1, space="PSUM") as psum_t,
            tc.tile_pool(name="psumBC", bufs=2, space="PSUM") as psum_bc,
        ):
            sb_xw = consts.tile([128, W], BF16)
            nc.scalar.dma_start(sb_xw[:], xw[:])
            sb_xrep = consts.tile([128, W], BF16 if GEN_FP16 else F32)
            nc.scalar.dma_start(sb_xrep[:], xrep[:])
            sb_ind2 = consts.tile([128, 32], BF16)
            nc.scalar.dma_start(sb_ind2[:], ind2[:])
            sb_ind2a = consts.tile([128, 64], BF16)
            nc.scalar.dma_start(sb_ind2a[:], ind2a[:])
            sb_ind2b = consts.tile([128, 64], BF16)
            nc.scalar.dma_start(sb_ind2b[:], ind2b[:])
            sb_psiYt = consts.tile([128, BPC, R], F32)
            nc.scalar.dma_start(sb_psiYt[:], psiYt[:])
            sb_psiXt = consts.tile([128, BPC, R], F32)
            nc.scalar.dma_start(sb_psiXt[:], psiXt[:])
            sb_wy2 = consts.tile([128, 32], F32)
            nc.scalar.dma_start(sb_wy2[:], wy2[:])
            sb_phicat = consts.tile([R, 2 * CO], F32)
            nc.scalar.dma_start(sb_phicat[:], phicat[:])
            sb_ybc = consts.tile([1, 384], F32)
            nc.scalar.dma_start(sb_ybc[:], ybc[:])
            sb_c128 = consts.tile([1, 2 * CO], F32)
            nc.scalar.dma_start(sb_c128[:], c128[:])
            sb_id1 = consts.tile([1, 1], F32)
            nc.scalar.dma_start(sb_id1[:], ident1[:])

            in_tiles = {}
            for b in range(BPC):
                for blk in range(NBLK):
                    t = in_pool.tile([128, 32, W], BF16, tag="in")
                    src = v[b, blk * IBLK:(blk + 1) * IBLK, :, :].rearrange(
                        "i (hb hl) w -> (i hb) hl w", hb=8
                    )
                    if INPUT_I8:
                        nc.gpsimd.dma_start(t[:], src)
                    else:
                        nc.sync.dma_start(t[:], src)
                    in_tiles[(b, blk)] = t

            CHUNK = 2
            NCH = 32 // CHUNK

            def reduce_batch(b):
                """PE hb-fold -> Qsb lines [128=(k,il,t), 32, W] -> trees -> SYX."""
                qsb = tree_pool.tile([128, 32, W], BF16, tag="qsb")
                for c in range(NCH):
                    qp = psum_q.tile([128, CHUNK, W], F32, tag="qp")
                    for blk in range(2):
                        nc.tensor.matmul(
                            qp[32 * blk:32 * (blk + 1), :, :], lhsT=sb_ind2[:],
                            rhs=in_tiles[(b, blk)][:, c * CHUNK:(c + 1) * CHUNK, :],
                            start=True, stop=True,
                        )
                    # blocks 2+3 share the [64:128] region (base-96 writes are
                    # not allowed): extended lhsT halves, accumulate pattern
                    nc.tensor.matmul(
                        qp[64:128, :, :], lhsT=sb_ind2a[:],
                        rhs=in_tiles[(b, 2)][:, c * CHUNK:(c + 1) * CHUNK, :],
                        start=True, stop=False,
                    )
                    nc.tensor.matmul(
                        qp[64:128, :, :], lhsT=sb_ind2b[:],
                        rhs=in_tiles[(b, 3)][:, c * CHUNK:(c + 1) * CHUNK, :],
                        start=False, stop=True,
                    )
                    dst = qsb[:, c * CHUNK:(c + 1) * CHUNK, :]
                    if c % 2 == 0:
                        nc.vector.tensor_copy(dst, qp[:])
                    else:
                        nc.scalar.copy(dst, qp[:])
                # w-halving tree (DVE bf16 2x) keeps hl resolution
                q = qsb
                wlen = W
                for lvl in range(5):  # 256 -> 8
                    wlen //= 2
                    qn = tree_pool.tile([128, 32, wlen], BF16, tag=f"q{lvl}")
                    nc.vector.tensor_tensor(
                        out=qn[:], in0=q[:, :, 0:wlen], in1=q[:, :, wlen:2 * wlen],
                        op=ADD,
                    )
                    q = qn
                cq = tree_pool.tile([128, 32], F32, tag="cq")
                nc.vector.tensor_reduce(out=cq[:], in_=q[:], axis=AXX, op=ADD)
                wq = tree_pool.tile([128, 32], F32, tag="wq")
                nc.vector.tensor_tensor(out=wq[:], in0=cq[:], in1=sb_wy2[:], op=MULT)
                syx = syx_pool.tile([128, 2], F32, tag="syx")
                nc.vector.tensor_reduce(out=syx[:, 0:1], in_=wq[:], axis=AXX, op=ADD)
                # h-halving tree (DVE bf16)
                a = qsb
                hlen = 32
                for lvl in range(5):  # 32 -> 1
                    hlen //= 2
                    an = tree_pool.tile([128, hlen, W], BF16, tag=f"a{lvl}")
                    nc.vector.tensor_tensor(
                        out=an[:], in0=a[:, 0:hlen, :], in1=a[:, hlen:2 * hlen, :],
                        op=ADD,
                    )
                    a = an
                wrs = tree_pool.tile([128, W], F32, tag="wrs")
                nc.vector.tensor_tensor(out=wrs[:], in0=a[:, 0, :], in1=sb_xw[:], op=MULT)
                nc.vector.tensor_reduce(out=syx[:, 1:2], in_=wrs[:], axis=AXX, op=ADD)
                return syx

            def tiny_rest(b, syx):
                """syx [128(il,hb), (k,yx)] partials -> bias/scale tiles for gen."""
                inner_ps = psum_t.tile([1, R], F32, tag="tiny")
                nc.tensor.matmul(
                    inner_ps[:], lhsT=syx[:, 0:1], rhs=sb_psiYt[:, b, :],
                    start=True, stop=False,
                )
                nc.tensor.matmul(
                    inner_ps[:], lhsT=syx[:, 1:2], rhs=sb_psiXt[:, b, :],
                    start=False, stop=True,
                )
                inner_sb = small.tile([1, R], F32, tag="ti1")
                nc.vector.tensor_copy(inner_sb[:], inner_ps[:])

                innT_ps = psum_t.tile([R, 1], F32, tag="tiny2")
                nc.tensor.transpose(innT_ps[:], inner_sb[:], sb_id1[:])
                innT = small.tile([R, 1], F32, tag="ti2")
                nc.vector.tensor_copy(innT[:], innT_ps[:])

                ab_ps = psum_t.tile([1, 2 * CO], F32, tag="tiny")
                nc.tensor.matmul(
                    ab_ps[:], lhsT=innT[:], rhs=sb_phicat[:], start=True, stop=True
                )
                ab = small.tile([1, 2 * CO], F32, tag="ti3")
                nc.vector.tensor_copy(ab[:], ab_ps[:])

                # per-channel scale s = (|A|+|B|)/127, inv, and scaled A,B
                absab = small.tile([1, 2 * CO], F32, tag="ti4")
                nc.scalar.activation(absab[:], ab[:], mybir.ActivationFunctionType.Abs)
                av = absab[:].rearrange("a (o t) -> a o t", t=2)
                s127 = small.tile([1, CO], F32, tag="ti5")
                nc.vector.tensor_tensor(
                    out=s127[:].unsqueeze(2), in0=av[:, :, 0:1], in1=av[:, :, 1:2],
                    op=ADD,
                )
                nc.vector.tensor_scalar(
                    out=s127[:], in0=s127[:], scalar1=1.0 / 126.0, scalar2=None,
                    op0=MULT,
                )
                nc.scalar.dma_start(s_out[b:b + 1, :], s127[:])
                invs = small.tile([1, CO], F32, tag="ti6")
                nc.vector.reciprocal(invs[:], s127[:])
                abq = small.tile([1, 2 * CO], F32, tag="ti7")
                nc.vector.tensor_tensor(
                    out=abq[:].rearrange("a (o t) -> a o t", t=2),
                    in0=ab[:].rearrange("a (o t) -> a o t", t=2),
                    in1=invs[:].unsqueeze(2).broadcast_to([1, CO, 2]),
                    op=MULT,
                )

                outs = []
                for k in range(3):  # bias_even, bias_odd, scale
                    ps = psum_bc.tile([128, 2 * CO], F32, tag="bc")
                    nc.tensor.matmul(
                        ps[:], lhsT=sb_ybc[0:1, 128 * k:128 * (k + 1)], rhs=abq[:],
                        start=True, stop=(k == 2),
                    )
                    if k < 2:  # + 128 offset on bias tiles
                        nc.tensor.matmul(
                            ps[:], lhsT=sb_ybc[0:1, 256:384], rhs=sb_c128[:],
                            start=False, stop=True,
                        )
                    sb = bc_pool.tile([128, 2 * CO], F32, tag="bcs")
                    if k % 2 == 0:
                        nc.scalar.copy(sb[:], ps[:])
                    else:
                        nc.vector.tensor_copy(sb[:], ps[:])
                    outs.append(sb)
                return outs  # [bias_even, bias_odd, scale]

            def gen_batch(b, bias_e, bias_o, scale):
                eng = 0
                for oc in range(NOBLK):
                    ot = out_pool.tile([128, OBLK, 2, W], BF16 if GEN_FP16 else U8, tag="out")
                    for ol in range(OBLK):
                        o = oc * OBLK + ol
                        sc_ap = scale[:, 2 * o + 1:2 * o + 2]
                        for hh in range(2):
                            bias_ap = (bias_e if hh == 0 else bias_o)[:, 2 * o:2 * o + 1]
                            dst = ot[:, ol, hh, :]
                            which = _GEN_ENGINES[eng % len(_GEN_ENGINES)]
                            eng += 1
                            if which == "dve":
                                nc.vector.tensor_scalar(
                                    out=dst, in0=sb_xrep[:], scalar1=sc_ap,
                                    scalar2=bias_ap, op0=MULT, op1=ADD,
                                )
                            elif which == "pool":
                                nc.gpsimd.tensor_scalar(
                                    out=dst, in0=sb_xrep[:], scalar1=sc_ap,
                                    scalar2=bias_ap, op0=MULT, op1=ADD,
                                )
                            else:
                                nc.scalar.activation(
                                    dst, sb_xrep[:], IDENT,
                                    bias=bias_ap, scale=sc_ap,
                                )
                    dma_eng = nc.gpsimd if GEN_FP16 else nc.scalar
                    dma_eng.dma_start(
                        u8o[b, oc * OBLK:(oc + 1) * OBLK, :, :].rearrange(
                            "o (p hh) w -> p o hh w", p=128
                        ),
                        ot[:],
                    )

            syx0 = reduce_batch(0)
            bc0 = tiny_rest(0, syx0)
            syx1 = reduce_batch(1)
            gen_batch(0, *bc0)
            bc1 = tiny_rest(1, syx1)
            gen_batch(1, *bc1)

    nc.compile()
    return nc


def make_in_maps(v, psi, phi):
    y = np.linspace(-1.0, 1.0, H, dtype=np.float64)
    x = np.linspace(-1.0, 1.0, W, dtype=np.float64)
    dx = 2.0 / (W - 1)
    dy = 2.0 / (H - 1)
    bf = ml_dtypes.bfloat16

    p = np.arange(128)
    pp = np.arange(128)  # line index p' = 32k + 2il + t
    il_of = (pp % 32) // 2
    k_of = pp // 32
    t_of = pp % 2
    chan = 16 * k_of + il_of  # [128]

    ind2 = np.zeros((128, 32), np.float64)
    ind2[p, 2 * (p // 8)] = 1.0
    ind2[p, 2 * (p // 8) + 1] = y[32 * (p % 8)]
    ind2a = np.zeros((128, 64), np.float64)
    ind2a[:, 0:32] = ind2
    ind2b = np.zeros((128, 64), np.float64)
    ind2b[:, 32:64] = ind2

    wy2 = np.where(t_of[:, None] == 1, 1.0, dy * np.arange(32)[None, :])

    psiY = np.ascontiguousarray(psi[:, :, 0].T * dx).astype(np.float64)  # [i, r]
    psiX = np.ascontiguousarray(psi[:, :, 1].T * dx).astype(np.float64)
    phicat = np.stack([phi[:, :, 0].T, phi[:, :, 1].T], axis=2).reshape(R, 2 * CO)
    ybc = np.concatenate([y[0::2], y[1::2], np.ones(128)])[None, :].astype(np.float32)
    c128 = np.full((1, 2 * CO), 128.0, dtype=np.float32)

    common = {
        "xw": np.ascontiguousarray(np.broadcast_to(x, (128, W))).astype(bf),
        "xrep": np.ascontiguousarray(np.broadcast_to(x, (128, W))).astype(
            bf if GEN_FP16 else np.float32),
        "ind2": ind2.astype(bf),
        "ind2a": ind2a.astype(bf),
        "ind2b": ind2b.astype(bf),
        "wy2": wy2.astype(np.float32),
        "phicat": np.ascontiguousarray(phicat).astype(np.float32),
        "ybc": ybc,
        "c128": c128,
        "ident1": np.ones((1, 1), dtype=np.float32),
    }

    def psit(sc_b):  # sc_b [BPC, CI] or None -> psiYt/psiXt [128, BPC, R]
        pyt = np.empty((128, BPC, R), np.float64)
        pxt = np.zeros((128, BPC, R), np.float64)
        for b in range(BPC):
            s = sc_b[b] if sc_b is not None else np.ones(CI)
            pyt[:, b, :] = psiY[chan] * s[chan][:, None]
            ev = t_of == 0
            pxt[ev, b, :] = psiX[chan[ev]] * s[chan[ev]][:, None]
        return pyt.astype(np.float32), pxt.astype(np.float32)

    if INPUT_I8:
        vf = v.reshape(N_CORES, BPC, CI, H, W)
        sc = np.abs(vf).max(axis=(3, 4)) / 127.0  # [cores, BPC, CI]
        q = np.rint(vf / sc[..., None, None]).astype(np.int8)
        in_maps = []
        for c in range(N_CORES):
            pyt, pxt = psit(sc[c])
            in_maps.append({"v": q[c], "psiYt": pyt, "psiXt": pxt, **common})
        return in_maps

    pyt, pxt = psit(None)
    common["psiYt"] = pyt
    common["psiXt"] = pxt
    shards = np.ascontiguousarray(v.astype(bf).reshape(N_CORES, BPC, CI, H, W))
    return [{"v": shards[i], **common} for i in range(N_CORES)]


_NC_CACHE = None


def kernel(v, psi, phi):
    global _NC_CACHE
    if _NC_CACHE is None:
        _NC_CACHE = build_nc()
    nc = _NC_CACHE
    in_maps = make_in_maps(
        np.ascontiguousarray(v, dtype=np.float32),
        np.asarray(psi, dtype=np.float32),
        np.asarray(phi, dtype=np.float32),
    )
    res = run_bass_kernel_spmd(nc, in_maps, core_ids=list(range(N_CORES)))
    return postprocess(res.results)


def postprocess(results):
    outs = []
    for r in results:
        u8 = r["u8o"].astype(np.float32)
        s = r["s_out"]  # [BPC, CO]
        u = (u8 - 128.0) * s[:, :, None, None]
        outs.append(u)
    return np.concatenate(outs, axis=0)


if __name__ == "__main__":
    build_nc()
    print("build ok")
